# revision 12
# baseline (speedup 1.0000x reference)
"""Trainium2 Bass kernel: per-pixel 5x5-patch channel covariance.

R[b,h,w,k,l] = (1/N) sum_n (p_kn - mu_k)(p_ln - mu_l)   (N=25, reflect pad)

Identity:  R = box5x5(S_k * S_l)/25 - mu_k * mu_l,  mu = box5x5(S)/25.

Device computes ONLY the 136 upper-triangle pair channels box5x5(S_k*S_l)/25
(host pre-scales S by 1/5 so two weight-1 banded box passes give /25).
Host computes mu (cheap separable box in fp32), subtracts mu_k*mu_l, and
mirrors the symmetric lower triangle -- all trivially vectorized numpy.

Dataflow per core (shard = one batch x one H-half, 128 out rows + halo):
  products (DVE, 4x mode)
    -> stage-1 row-box banded matmuls (TensorE, out = [w_chunk, 64h] psum)
    -> psum->sbuf copies into i1[w, ch, h] (split Act/Pool)
    -> stage-2 col-box matmuls lhsT=i1[:,c,:], rhs=BW chunk (out = [128h, 256w])
    -> psum->sbuf copies (split DVE/Act/Pool)
    -> DMA out channel-major [136, 128, 256] (512B descriptors)

Sharding: 8 cores = 4 batches x 2 H-halves.  Fully data parallel.
"""
import sys

sys.path.insert(0, "/opt/trn_rl_repo")

from contextlib import ExitStack

import numpy as np

import concourse.bacc as bacc
import concourse.mybir as mybir
import concourse.tile as tile
from concourse import bass_utils

B, K, H, W = 4, 16, 256, 256
HH = 128           # output rows per core
SR = 132           # shard rows (128 + 2 halo each side, edge-clamped)
NPAIR = K * (K + 1) // 2   # 136 upper-triangle channels
NOCT = NPAIR // 8          # 17 channel octets
F32 = mybir.dt.float32
BF16 = mybir.dt.bfloat16

# Pool (GPSIMD) cannot read PSUM, so all psum->sbuf copies go Act/DVE
# (2:1 by rate); Pool instead takes the last product octets off DVE,
# emitted as a pre-pass so they are ready when the main loop reaches them.
POOL_OCTS = {(0, 12), (0, 13), (0, 14), (0, 15), (0, 16),
             (1, 11), (1, 12), (1, 13), (1, 14), (1, 15), (1, 16)}


def _reflect_idx(i, n):
    if i < 0:
        return -i
    if i >= n:
        return 2 * (n - 1) - i
    return i


def _build_bw():
    """[128, 512] col-box weights, reflect folded: [:, c*256:(c+1)*256] =
    M[c*128:(c+1)*128, :] where M[w_src, w_out] is the 256x256 band."""
    M = np.zeros((W, W), dtype=np.float32)
    for w in range(W):
        for j in range(5):
            M[_reflect_idx(w - 2 + j, W), w] += 1.0
    out = np.zeros((128, 512), dtype=np.float32)
    out[:, 0:256] = M[0:128, :]
    out[:, 256:512] = M[128:256, :]
    return out


def _build_br(half):
    """[68, 128] row-box weights: cols rt*64+hl; rows shard-local within rt."""
    hbase = half * HH
    M = np.zeros((68, 128), dtype=np.float32)
    for rt in range(2):
        for hl in range(64):
            hg = hbase + rt * 64 + hl
            for i in range(5):
                r = _reflect_idx(hg - 2 + i, H)
                j = r + 2 - hbase          # canonical shard row
                M[j - rt * 64, rt * 64 + hl] += 1.0
    return M


def _ksegs_in_octet(oct_idx):
    """Pair channels 0..135 in (k outer, l=k..15) order. For channel octet
    [oct*8, oct*8+8) return (j0, k, l0, nl): local offset, k, first l, count."""
    lo, hi = oct_idx * 8, oct_idx * 8 + 8
    segs = []
    p = 0
    for k in range(K):
        n = K - k
        s, e = p, p + n
        a, b = max(lo, s), min(hi, e)
        if a < b:
            segs.append((a - lo, k, k + (a - s), b - a))
        p += n
    return segs


def _build_kernel():
    nc = bacc.Bacc("TRN2", target_bir_lowering=False, debug=False)
    S_d = nc.dram_tensor("S", [SR, K, W], BF16, kind="ExternalInput").ap()
    BR_d = nc.dram_tensor("BR", [68, 128], BF16, kind="ExternalInput").ap()
    BW_d = nc.dram_tensor("BW", [128, 512], BF16, kind="ExternalInput").ap()
    R_d = nc.dram_tensor("R", [NPAIR, HH, W], BF16, kind="ExternalOutput").ap()

    cp_eng = None  # set inside

    with tile.TileContext(nc) as tc, ExitStack() as ctx:
        const_p = ctx.enter_context(tc.tile_pool(name="const", bufs=1))
        sp_p = ctx.enter_context(tc.tile_pool(name="sp", bufs=1))
        t_p = ctx.enter_context(tc.tile_pool(name="tprod", bufs=10))
        tp_p = ctx.enter_context(tc.tile_pool(name="tpool", bufs=1))
        i1_p = ctx.enter_context(tc.tile_pool(name="i1", bufs=1))
        r_p = ctx.enter_context(tc.tile_pool(name="rout", bufs=3))
        ps1_p = ctx.enter_context(tc.tile_pool(name="ps1", bufs=2, space="PSUM"))
        ps2_p = ctx.enter_context(tc.tile_pool(name="ps2", bufs=2, space="PSUM"))

        ncopy = 0

        def copy_psum(dst, src):
            nonlocal ncopy
            if ncopy % 3 == 2:
                nc.vector.tensor_copy(dst, src)
            else:
                nc.scalar.copy(dst, src)
            ncopy += 1

        br = const_p.tile([68, 128], BF16)
        bw = const_p.tile([128, 512], BF16)
        nc.sync.dma_start(br[:], BR_d)
        nc.sync.dma_start(bw[:], BW_d)

        sp0 = sp_p.tile([68, K, W], BF16)
        sp1 = sp_p.tile([68, K, W], BF16)
        nc.sync.dma_start(sp0[:], S_d[0:68])
        nc.sync.dma_start(sp1[:], S_d[64:132])
        sps = [sp0, sp1]

        # i1[w_local, ch, chunk, h]: stage-1 output; chunk X covers w 128X..+128
        i1 = i1_p.tile([128, NPAIR, 2, HH], BF16, name="i1")

        def products(rt, oc, mul, pool):
            sp = sps[rt]
            T = pool.tile([68, 8, W], BF16, name=f"T{rt}_{oc}" if
                          pool is tp_p else "T")
            for (j0, k, l0, nl) in _ksegs_in_octet(oc):
                in0 = sp[:, k, :].unsqueeze(1).broadcast_to([68, nl, W])
                mul(T[:, j0:j0 + nl, :], in0, sp[:, l0:l0 + nl, :])
            return T

        # Pool pre-pass: last octets' products, interleaved rt0/rt1
        pool_T = {}
        for oc in range(NOCT - 1, 10, -1):
            for rt in range(2):
                if (rt, oc) in POOL_OCTS:
                    pool_T[(rt, oc)] = products(
                        rt, oc, nc.gpsimd.tensor_mul, tp_p)

        def prefetch_products(ocp):
            """Emit DVE products for group ocp ahead of older copies in the
            DVE queue, so products (critical path to PE) are not delayed."""
            if ocp > 8:
                return
            for oi in range(2 if ocp < 8 else 1):
                oc = ocp * 2 + oi
                for rt in range(2):
                    if (rt, oc) not in pool_T:
                        pool_T[(rt, oc)] = products(
                            rt, oc, nc.vector.tensor_mul, t_p)

        bw0 = bw[:, 0:256]
        bw1 = bw[:, 256:512]
        prefetch_products(0)
        for ocp in range(9):            # 16-channel groups (last is 8)
            nocts = 2 if ocp < 8 else 1
            prefetch_products(ocp + 1)
            # ---- stage 1: row box for this group's octets, both row-tiles
            for oi in range(nocts):
                oc = ocp * 2 + oi
                for rt in range(2):
                    brt = br[:, rt * 64:(rt + 1) * 64]
                    T = pool_T[(rt, oc)]
                    Tf = T[:].rearrange("p a b -> p (a b)")
                    ps1 = ps1_p.tile([128, 1024], F32, name="ps1")
                    for j in range(8):
                        nc.tensor.matmul(ps1[:, j * 128:j * 128 + 64],
                                         Tf[:, j * 256:j * 256 + 128], brt,
                                         start=True, stop=True)
                        nc.tensor.matmul(ps1[:, j * 128 + 64:j * 128 + 128],
                                         Tf[:, j * 256 + 128:j * 256 + 256],
                                         brt, start=True, stop=True)
                    copy_psum(
                        i1[:, oc * 8:(oc + 1) * 8, :, rt * 64:(rt + 1) * 64],
                        ps1[:].rearrange("p (c k h) -> p c k h", c=8, k=2))
            # ---- stage 2: col box -> [128 h, 256 w] per channel
            c0, nch = ocp * 16, 8 * nocts
            rsb = r_p.tile([128, 16, W], BF16, name="rsb")
            for cq in range(nch // 4):
                ps2 = ps2_p.tile([128, 1024], F32, name="ps2")
                for ci in range(4):
                    c = c0 + cq * 4 + ci
                    nc.tensor.matmul(ps2[:, ci * 256:(ci + 1) * 256],
                                     i1[:, c, 0, :], bw0,
                                     start=True, stop=False)
                    nc.tensor.matmul(ps2[:, ci * 256:(ci + 1) * 256],
                                     i1[:, c, 1, :], bw1,
                                     start=False, stop=True)
                copy_psum(rsb[:, cq * 4:cq * 4 + 4, :],
                          ps2[:].rearrange("p (c w) -> p c w", c=4))
            dview = R_d[c0:c0 + nch, :, :].transpose([1, 0, 2])
            nc.sync.dma_start(dview, rsb[:, 0:nch, :])

    nc.compile()
    return nc


_NC_CACHE = {}


def _get_nc():
    if "nc" not in _NC_CACHE:
        _NC_CACHE["nc"] = _build_kernel()
    return _NC_CACHE["nc"]


def _prep_in_maps(S):
    S = np.asarray(S, dtype=np.float32)
    np_bf16 = mybir.dt.np(BF16)
    bw = _build_bw().astype(np_bf16)
    brs = [(_build_br(h)).astype(np_bf16) for h in range(2)]
    Ss = S * np.float32(0.2)
    in_maps = []
    for b in range(B):
        for half in range(2):
            hbase = half * HH
            rows = np.clip(np.arange(hbase - 2, hbase + 130), 0, H - 1)
            shard = Ss[b][:, rows, :].transpose(1, 0, 2)   # [132, K, 256]
            shard = np.ascontiguousarray(shard).astype(np_bf16)
            in_maps.append({"S": shard, "BR": brs[half], "BW": bw})
    return in_maps


def _box25(x):
    """Separable 5x5 box sum with reflect padding over last two axes."""
    xp = np.pad(x, ((0, 0), (0, 0), (2, 2), (2, 2)), mode="reflect")
    yh = xp[:, :, 0:H, :].copy()
    for i in range(1, 5):
        yh += xp[:, :, i:i + H, :]
    y = yh[:, :, :, 0:W].copy()
    for j in range(1, 5):
        y += yh[:, :, :, j:j + W]
    return y


def _assemble(results, S):
    iu, il = np.triu_indices(K)            # same order as device channels
    mu = _box25(np.asarray(S, np.float32)) * np.float32(1.0 / 25.0)
    out = np.empty((B, H, W, K, K), dtype=np.float32)
    for i in range(8):
        b, half = divmod(i, 2)
        hs = slice(half * HH, (half + 1) * HH)
        r = np.asarray(results[i]["R"]).astype(np.float32)   # [136, 128, 256]
        v = r - mu[b, iu, hs, :] * mu[b, il, hs, :]          # [136, 128, 256]
        v = np.moveaxis(v, 0, -1)                            # [128, 256, 136]
        flat = np.empty((HH, W, K * K), dtype=np.float32)
        flat[..., iu * K + il] = v
        flat[..., il * K + iu] = v
        out[b, hs] = flat.reshape(HH, W, K, K)
    return out


def kernel(S):
    """S: [4, 16, 256, 256] float32 -> R: [4, 256, 256, 16, 16] float32."""
    nc = _get_nc()
    in_maps = _prep_in_maps(S)
    res = bass_utils.run_bass_kernel_spmd(nc, in_maps, list(range(8)))
    return _assemble(res.results, S)


# revision 13
# speedup vs baseline: 1.0011x; 1.0011x over previous
"""Trainium2 Bass kernel: per-pixel 5x5-patch channel covariance.

R[b,h,w,k,l] = (1/N) sum_n (p_kn - mu_k)(p_ln - mu_l)   (N=25, reflect pad)

Identity:  R = box5x5(S_k * S_l)/25 - mu_k * mu_l,  mu = box5x5(S)/25.

Device computes ONLY the 136 upper-triangle pair channels box5x5(S_k*S_l)/25
(host pre-scales S by 1/5 so two weight-1 banded box passes give /25).
Host computes mu (cheap separable box in fp32), subtracts mu_k*mu_l, and
mirrors the symmetric lower triangle -- all trivially vectorized numpy.

Dataflow per core (shard = one batch x one H-half, 128 out rows + halo):
  products (DVE, 4x mode)
    -> stage-1 row-box banded matmuls (TensorE, out = [w_chunk, 64h] psum)
    -> psum->sbuf copies into i1[w, ch, h] (split Act/Pool)
    -> stage-2 col-box matmuls lhsT=i1[:,c,:], rhs=BW chunk (out = [128h, 256w])
    -> psum->sbuf copies (split DVE/Act/Pool)
    -> DMA out channel-major [136, 128, 256] (512B descriptors)

Sharding: 8 cores = 4 batches x 2 H-halves.  Fully data parallel.
"""
import sys

sys.path.insert(0, "/opt/trn_rl_repo")

from contextlib import ExitStack

import numpy as np

import concourse.bacc as bacc
import concourse.mybir as mybir
import concourse.tile as tile
from concourse import bass_utils

B, K, H, W = 4, 16, 256, 256
HH = 128           # output rows per core
SR = 132           # shard rows (128 + 2 halo each side, edge-clamped)
NPAIR = K * (K + 1) // 2   # 136 upper-triangle channels
NOCT = NPAIR // 8          # 17 channel octets
F32 = mybir.dt.float32
BF16 = mybir.dt.bfloat16

# Pool (GPSIMD) cannot read PSUM, so all psum->sbuf copies go Act/DVE
# (2:1 by rate); Pool instead takes the last product octets off DVE,
# emitted as a pre-pass so they are ready when the main loop reaches them.
POOL_OCTS = {(0, 12), (0, 13), (0, 14), (0, 15), (0, 16),
             (1, 11), (1, 12), (1, 13), (1, 14), (1, 15), (1, 16)}


def _reflect_idx(i, n):
    if i < 0:
        return -i
    if i >= n:
        return 2 * (n - 1) - i
    return i


def _build_bw():
    """[128, 512] col-box weights, reflect folded: [:, c*256:(c+1)*256] =
    M[c*128:(c+1)*128, :] where M[w_src, w_out] is the 256x256 band."""
    M = np.zeros((W, W), dtype=np.float32)
    for w in range(W):
        for j in range(5):
            M[_reflect_idx(w - 2 + j, W), w] += 1.0
    out = np.zeros((128, 512), dtype=np.float32)
    out[:, 0:256] = M[0:128, :]
    out[:, 256:512] = M[128:256, :]
    return out


def _build_br(half):
    """[68, 128] row-box weights: cols rt*64+hl; rows shard-local within rt."""
    hbase = half * HH
    M = np.zeros((68, 128), dtype=np.float32)
    for rt in range(2):
        for hl in range(64):
            hg = hbase + rt * 64 + hl
            for i in range(5):
                r = _reflect_idx(hg - 2 + i, H)
                j = r + 2 - hbase          # canonical shard row
                M[j - rt * 64, rt * 64 + hl] += 1.0
    return M


def _ksegs_in_octet(oct_idx):
    """Pair channels 0..135 in (k outer, l=k..15) order. For channel octet
    [oct*8, oct*8+8) return (j0, k, l0, nl): local offset, k, first l, count."""
    lo, hi = oct_idx * 8, oct_idx * 8 + 8
    segs = []
    p = 0
    for k in range(K):
        n = K - k
        s, e = p, p + n
        a, b = max(lo, s), min(hi, e)
        if a < b:
            segs.append((a - lo, k, k + (a - s), b - a))
        p += n
    return segs


def _build_kernel():
    nc = bacc.Bacc("TRN2", target_bir_lowering=False, debug=False)
    S_d = nc.dram_tensor("S", [SR, K, W], BF16, kind="ExternalInput").ap()
    BR_d = nc.dram_tensor("BR", [68, 128], BF16, kind="ExternalInput").ap()
    BW_d = nc.dram_tensor("BW", [128, 512], BF16, kind="ExternalInput").ap()
    R_d = nc.dram_tensor("R", [NPAIR, HH, W], BF16, kind="ExternalOutput").ap()

    cp_eng = None  # set inside

    with tile.TileContext(nc) as tc, ExitStack() as ctx:
        const_p = ctx.enter_context(tc.tile_pool(name="const", bufs=1))
        sp_p = ctx.enter_context(tc.tile_pool(name="sp", bufs=1))
        t_p = ctx.enter_context(tc.tile_pool(name="tprod", bufs=10))
        tp_p = ctx.enter_context(tc.tile_pool(name="tpool", bufs=1))
        i1_p = ctx.enter_context(tc.tile_pool(name="i1", bufs=1))
        r_p = ctx.enter_context(tc.tile_pool(name="rout", bufs=3))
        ps1_p = ctx.enter_context(tc.tile_pool(name="ps1", bufs=2, space="PSUM"))
        ps2_p = ctx.enter_context(tc.tile_pool(name="ps2", bufs=2, space="PSUM"))

        ncopy = 0

        def copy_psum(dst, src):
            nonlocal ncopy
            if ncopy % 3 == 2:
                nc.vector.tensor_copy(dst, src)
            else:
                nc.scalar.copy(dst, src)
            ncopy += 1

        br = const_p.tile([68, 128], BF16)
        bw = const_p.tile([128, 512], BF16)
        nc.sync.dma_start(br[:], BR_d)
        nc.sync.dma_start(bw[:], BW_d)

        sp0 = sp_p.tile([68, K, W], BF16)
        sp1 = sp_p.tile([68, K, W], BF16)
        nc.sync.dma_start(sp0[:], S_d[0:68])
        nc.sync.dma_start(sp1[:], S_d[64:132])
        sps = [sp0, sp1]

        # i1[w_local, ch, chunk, h]: stage-1 output; chunk X covers w 128X..+128
        i1 = i1_p.tile([128, NPAIR, 2, HH], BF16, name="i1")

        def products(rt, oc, mul, pool):
            sp = sps[rt]
            T = pool.tile([68, 8, W], BF16, name=f"T{rt}_{oc}" if
                          pool is tp_p else "T")
            for (j0, k, l0, nl) in _ksegs_in_octet(oc):
                in0 = sp[:, k, :].unsqueeze(1).broadcast_to([68, nl, W])
                mul(T[:, j0:j0 + nl, :], in0, sp[:, l0:l0 + nl, :])
            return T

        # Pool pre-pass: last octets' products, interleaved rt0/rt1
        pool_T = {}
        for oc in range(11, NOCT):
            for rt in range(2):
                if (rt, oc) in POOL_OCTS:
                    pool_T[(rt, oc)] = products(
                        rt, oc, nc.gpsimd.tensor_mul, tp_p)

        def prefetch_products(ocp):
            """Emit DVE products for group ocp ahead of older copies in the
            DVE queue, so products (critical path to PE) are not delayed."""
            if ocp > 8:
                return
            for oi in range(2 if ocp < 8 else 1):
                oc = ocp * 2 + oi
                for rt in range(2):
                    if (rt, oc) not in pool_T:
                        pool_T[(rt, oc)] = products(
                            rt, oc, nc.vector.tensor_mul, t_p)

        bw0 = bw[:, 0:256]
        bw1 = bw[:, 256:512]
        prefetch_products(0)
        for ocp in range(9):            # 16-channel groups (last is 8)
            nocts = 2 if ocp < 8 else 1
            prefetch_products(ocp + 1)
            # ---- stage 1: row box for this group's octets, both row-tiles
            for oi in range(nocts):
                oc = ocp * 2 + oi
                for rt in range(2):
                    brt = br[:, rt * 64:(rt + 1) * 64]
                    T = pool_T[(rt, oc)]
                    Tf = T[:].rearrange("p a b -> p (a b)")
                    ps1 = ps1_p.tile([128, 1024], F32, name="ps1")
                    for j in range(8):
                        nc.tensor.matmul(ps1[:, j * 128:j * 128 + 64],
                                         Tf[:, j * 256:j * 256 + 128], brt,
                                         start=True, stop=True)
                        nc.tensor.matmul(ps1[:, j * 128 + 64:j * 128 + 128],
                                         Tf[:, j * 256 + 128:j * 256 + 256],
                                         brt, start=True, stop=True)
                    copy_psum(
                        i1[:, oc * 8:(oc + 1) * 8, :, rt * 64:(rt + 1) * 64],
                        ps1[:].rearrange("p (c k h) -> p c k h", c=8, k=2))
            # ---- stage 2: col box -> [128 h, 256 w] per channel
            c0, nch = ocp * 16, 8 * nocts
            rsb = r_p.tile([128, 16, W], BF16, name="rsb")
            for cq in range(nch // 4):
                ps2 = ps2_p.tile([128, 1024], F32, name="ps2")
                for ci in range(4):
                    c = c0 + cq * 4 + ci
                    nc.tensor.matmul(ps2[:, ci * 256:(ci + 1) * 256],
                                     i1[:, c, 0, :], bw0,
                                     start=True, stop=False)
                    nc.tensor.matmul(ps2[:, ci * 256:(ci + 1) * 256],
                                     i1[:, c, 1, :], bw1,
                                     start=False, stop=True)
                copy_psum(rsb[:, cq * 4:cq * 4 + 4, :],
                          ps2[:].rearrange("p (c w) -> p c w", c=4))
            dview = R_d[c0:c0 + nch, :, :].transpose([1, 0, 2])
            nc.sync.dma_start(dview, rsb[:, 0:nch, :])

    nc.compile()
    return nc


_NC_CACHE = {}


def _get_nc():
    if "nc" not in _NC_CACHE:
        _NC_CACHE["nc"] = _build_kernel()
    return _NC_CACHE["nc"]


def _prep_in_maps(S):
    S = np.asarray(S, dtype=np.float32)
    np_bf16 = mybir.dt.np(BF16)
    bw = _build_bw().astype(np_bf16)
    brs = [(_build_br(h)).astype(np_bf16) for h in range(2)]
    Ss = S * np.float32(0.2)
    in_maps = []
    for b in range(B):
        for half in range(2):
            hbase = half * HH
            rows = np.clip(np.arange(hbase - 2, hbase + 130), 0, H - 1)
            shard = Ss[b][:, rows, :].transpose(1, 0, 2)   # [132, K, 256]
            shard = np.ascontiguousarray(shard).astype(np_bf16)
            in_maps.append({"S": shard, "BR": brs[half], "BW": bw})
    return in_maps


def _box25(x):
    """Separable 5x5 box sum with reflect padding over last two axes."""
    xp = np.pad(x, ((0, 0), (0, 0), (2, 2), (2, 2)), mode="reflect")
    yh = xp[:, :, 0:H, :].copy()
    for i in range(1, 5):
        yh += xp[:, :, i:i + H, :]
    y = yh[:, :, :, 0:W].copy()
    for j in range(1, 5):
        y += yh[:, :, :, j:j + W]
    return y


def _assemble(results, S):
    iu, il = np.triu_indices(K)            # same order as device channels
    mu = _box25(np.asarray(S, np.float32)) * np.float32(1.0 / 25.0)
    out = np.empty((B, H, W, K, K), dtype=np.float32)
    for i in range(8):
        b, half = divmod(i, 2)
        hs = slice(half * HH, (half + 1) * HH)
        r = np.asarray(results[i]["R"]).astype(np.float32)   # [136, 128, 256]
        v = r - mu[b, iu, hs, :] * mu[b, il, hs, :]          # [136, 128, 256]
        v = np.moveaxis(v, 0, -1)                            # [128, 256, 136]
        flat = np.empty((HH, W, K * K), dtype=np.float32)
        flat[..., iu * K + il] = v
        flat[..., il * K + iu] = v
        out[b, hs] = flat.reshape(HH, W, K, K)
    return out


def kernel(S):
    """S: [4, 16, 256, 256] float32 -> R: [4, 256, 256, 16, 16] float32."""
    nc = _get_nc()
    in_maps = _prep_in_maps(S)
    res = bass_utils.run_bass_kernel_spmd(nc, in_maps, list(range(8)))
    return _assemble(res.results, S)


# revision 15
# speedup vs baseline: 1.0722x; 1.0709x over previous
"""Trainium2 Bass kernel: per-pixel 5x5-patch channel covariance.

R[b,h,w,k,l] = (1/N) sum_n (p_kn - mu_k)(p_ln - mu_l)   (N=25, reflect pad)

Identity:  R = box5x5(S_k * S_l)/25 - mu_k * mu_l,  mu = box5x5(S)/25.

Device computes ONLY the 136 upper-triangle pair channels box5x5(S_k*S_l)/25
(host pre-scales S by 1/5 so two weight-1 banded box passes give /25).
Host computes mu (cheap separable box in fp32), subtracts mu_k*mu_l, and
mirrors the symmetric lower triangle -- all trivially vectorized numpy.

Dataflow per core (shard = one batch x one H-half, 128 out rows + halo):
  products (DVE, 4x mode)
    -> stage-1 row-box banded matmuls (TensorE, out = [w_chunk, 64h] psum)
    -> psum->sbuf copies into i1[w, ch, h] (split Act/Pool)
    -> stage-2 col-box matmuls lhsT=i1[:,c,:], rhs=BW chunk (out = [128h, 256w])
    -> psum->sbuf copies (split DVE/Act/Pool)
    -> DMA out channel-major [136, 128, 256] (512B descriptors)

Sharding: 8 cores = 4 batches x 2 H-halves.  Fully data parallel.
"""
import sys

sys.path.insert(0, "/opt/trn_rl_repo")

from contextlib import ExitStack

import numpy as np

import concourse.bacc as bacc
import concourse.mybir as mybir
import concourse.tile as tile
from concourse import bass_utils

B, K, H, W = 4, 16, 256, 256
HH = 128           # output rows per core
SR = 132           # shard rows (128 + 2 halo each side, edge-clamped)
NPAIR = K * (K + 1) // 2   # 136 upper-triangle channels
NOCT = NPAIR // 8          # 17 channel octets
F32 = mybir.dt.float32
BF16 = mybir.dt.bfloat16

# Pool (GPSIMD) cannot read PSUM, so all psum->sbuf copies go Act/DVE
# (2:1 by rate); Pool instead takes the last product octets off DVE,
# emitted as a pre-pass so they are ready when the main loop reaches them.
# Ordered: emitted (and consumed) in this sequence; spread so one product
# octet per channel-group comes from Pool throughout the run.
POOL_OCTS = [(0, 1), (0, 3), (0, 5), (0, 7), (0, 9), (0, 11),
             (0, 13), (1, 13), (0, 15), (1, 15), (1, 16)]


def _reflect_idx(i, n):
    if i < 0:
        return -i
    if i >= n:
        return 2 * (n - 1) - i
    return i


def _build_bw():
    """[128, 512] col-box weights, reflect folded: [:, c*256:(c+1)*256] =
    M[c*128:(c+1)*128, :] where M[w_src, w_out] is the 256x256 band."""
    M = np.zeros((W, W), dtype=np.float32)
    for w in range(W):
        for j in range(5):
            M[_reflect_idx(w - 2 + j, W), w] += 1.0
    out = np.zeros((128, 512), dtype=np.float32)
    out[:, 0:256] = M[0:128, :]
    out[:, 256:512] = M[128:256, :]
    return out


def _build_br(half):
    """[68, 128] row-box weights: cols rt*64+hl; rows shard-local within rt."""
    hbase = half * HH
    M = np.zeros((68, 128), dtype=np.float32)
    for rt in range(2):
        for hl in range(64):
            hg = hbase + rt * 64 + hl
            for i in range(5):
                r = _reflect_idx(hg - 2 + i, H)
                j = r + 2 - hbase          # canonical shard row
                M[j - rt * 64, rt * 64 + hl] += 1.0
    return M


def _ksegs_in_octet(oct_idx):
    """Pair channels 0..135 in (k outer, l=k..15) order. For channel octet
    [oct*8, oct*8+8) return (j0, k, l0, nl): local offset, k, first l, count."""
    lo, hi = oct_idx * 8, oct_idx * 8 + 8
    segs = []
    p = 0
    for k in range(K):
        n = K - k
        s, e = p, p + n
        a, b = max(lo, s), min(hi, e)
        if a < b:
            segs.append((a - lo, k, k + (a - s), b - a))
        p += n
    return segs


def _build_kernel():
    nc = bacc.Bacc("TRN2", target_bir_lowering=False, debug=False)
    S_d = nc.dram_tensor("S", [SR, K, W], BF16, kind="ExternalInput").ap()
    BR_d = nc.dram_tensor("BR", [68, 128], BF16, kind="ExternalInput").ap()
    BW_d = nc.dram_tensor("BW", [128, 512], BF16, kind="ExternalInput").ap()
    R_d = nc.dram_tensor("R", [NPAIR, HH, W], BF16, kind="ExternalOutput").ap()

    cp_eng = None  # set inside

    with tile.TileContext(nc) as tc, ExitStack() as ctx:
        const_p = ctx.enter_context(tc.tile_pool(name="const", bufs=1))
        sp_p = ctx.enter_context(tc.tile_pool(name="sp", bufs=1))
        t_p = ctx.enter_context(tc.tile_pool(name="tprod", bufs=10))
        tp_p = ctx.enter_context(tc.tile_pool(name="tpool", bufs=1))
        i1_p = ctx.enter_context(tc.tile_pool(name="i1", bufs=1))
        r_p = ctx.enter_context(tc.tile_pool(name="rout", bufs=3))
        ps1_p = ctx.enter_context(tc.tile_pool(name="ps1", bufs=2, space="PSUM"))
        ps2_p = ctx.enter_context(tc.tile_pool(name="ps2", bufs=2, space="PSUM"))

        ncopy = 0

        def copy_psum(dst, src):
            nonlocal ncopy
            if ncopy % 3 == 2:
                nc.vector.tensor_copy(dst, src)
            else:
                nc.scalar.copy(dst, src)
            ncopy += 1

        br = const_p.tile([68, 128], BF16)
        bw = const_p.tile([128, 512], BF16)
        nc.sync.dma_start(br[:], BR_d)
        nc.sync.dma_start(bw[:], BW_d)

        sp0 = sp_p.tile([68, K, W], BF16)
        sp1 = sp_p.tile([68, K, W], BF16)
        nc.sync.dma_start(sp0[:], S_d[0:68])
        nc.sync.dma_start(sp1[:], S_d[64:132])
        sps = [sp0, sp1]

        # i1[w_local, ch, chunk, h]: stage-1 output; chunk X covers w 128X..+128
        i1 = i1_p.tile([128, NPAIR, 2, HH], BF16, name="i1")

        def products(rt, oc, mul, pool):
            sp = sps[rt]
            T = pool.tile([68, 8, W], BF16, name=f"T{rt}_{oc}" if
                          pool is tp_p else "T")
            for (j0, k, l0, nl) in _ksegs_in_octet(oc):
                in0 = sp[:, k, :].unsqueeze(1).broadcast_to([68, nl, W])
                mul(T[:, j0:j0 + nl, :], in0, sp[:, l0:l0 + nl, :])
            return T

        # Pool pre-pass: spread product octets, in consumption order
        pool_T = {}
        for (rt, oc) in POOL_OCTS:
            pool_T[(rt, oc)] = products(rt, oc, nc.gpsimd.tensor_mul, tp_p)

        def prefetch_products(ocp):
            """Emit DVE products for group ocp ahead of older copies in the
            DVE queue, so products (critical path to PE) are not delayed."""
            if ocp > 8:
                return
            for oi in range(2 if ocp < 8 else 1):
                oc = ocp * 2 + oi
                for rt in range(2):
                    if (rt, oc) not in pool_T:
                        pool_T[(rt, oc)] = products(
                            rt, oc, nc.vector.tensor_mul, t_p)

        bw0 = bw[:, 0:256]
        bw1 = bw[:, 256:512]
        prefetch_products(0)
        for ocp in range(9):            # 16-channel groups (last is 8)
            nocts = 2 if ocp < 8 else 1
            prefetch_products(ocp + 1)
            # ---- stage 1: row box for this group's octets, both row-tiles
            for oi in range(nocts):
                oc = ocp * 2 + oi
                for rt in range(2):
                    brt = br[:, rt * 64:(rt + 1) * 64]
                    T = pool_T[(rt, oc)]
                    Tf = T[:].rearrange("p a b -> p (a b)")
                    ps1 = ps1_p.tile([128, 1024], F32, name="ps1")
                    for j in range(8):
                        nc.tensor.matmul(ps1[:, j * 128:j * 128 + 64],
                                         Tf[:, j * 256:j * 256 + 128], brt,
                                         start=True, stop=True)
                        nc.tensor.matmul(ps1[:, j * 128 + 64:j * 128 + 128],
                                         Tf[:, j * 256 + 128:j * 256 + 256],
                                         brt, start=True, stop=True)
                    copy_psum(
                        i1[:, oc * 8:(oc + 1) * 8, :, rt * 64:(rt + 1) * 64],
                        ps1[:].rearrange("p (c k h) -> p c k h", c=8, k=2))
            # ---- stage 2: col box -> [128 h, 256 w] per channel
            c0, nch = ocp * 16, 8 * nocts
            rsb = r_p.tile([128, 16, W], BF16, name="rsb")
            for cq in range(nch // 4):
                ps2 = ps2_p.tile([128, 1024], F32, name="ps2")
                for ci in range(4):
                    c = c0 + cq * 4 + ci
                    nc.tensor.matmul(ps2[:, ci * 256:(ci + 1) * 256],
                                     i1[:, c, 0, :], bw0,
                                     start=True, stop=False)
                    nc.tensor.matmul(ps2[:, ci * 256:(ci + 1) * 256],
                                     i1[:, c, 1, :], bw1,
                                     start=False, stop=True)
                copy_psum(rsb[:, cq * 4:cq * 4 + 4, :],
                          ps2[:].rearrange("p (c w) -> p c w", c=4))
            dview = R_d[c0:c0 + nch, :, :].transpose([1, 0, 2])
            nc.sync.dma_start(dview, rsb[:, 0:nch, :])

    nc.compile()
    return nc


_NC_CACHE = {}


def _get_nc():
    if "nc" not in _NC_CACHE:
        _NC_CACHE["nc"] = _build_kernel()
    return _NC_CACHE["nc"]


def _prep_in_maps(S):
    S = np.asarray(S, dtype=np.float32)
    np_bf16 = mybir.dt.np(BF16)
    bw = _build_bw().astype(np_bf16)
    brs = [(_build_br(h)).astype(np_bf16) for h in range(2)]
    Ss = S * np.float32(0.2)
    in_maps = []
    for b in range(B):
        for half in range(2):
            hbase = half * HH
            rows = np.clip(np.arange(hbase - 2, hbase + 130), 0, H - 1)
            shard = Ss[b][:, rows, :].transpose(1, 0, 2)   # [132, K, 256]
            shard = np.ascontiguousarray(shard).astype(np_bf16)
            in_maps.append({"S": shard, "BR": brs[half], "BW": bw})
    return in_maps


def _box25(x):
    """Separable 5x5 box sum with reflect padding over last two axes."""
    xp = np.pad(x, ((0, 0), (0, 0), (2, 2), (2, 2)), mode="reflect")
    yh = xp[:, :, 0:H, :].copy()
    for i in range(1, 5):
        yh += xp[:, :, i:i + H, :]
    y = yh[:, :, :, 0:W].copy()
    for j in range(1, 5):
        y += yh[:, :, :, j:j + W]
    return y


def _assemble(results, S):
    iu, il = np.triu_indices(K)            # same order as device channels
    mu = _box25(np.asarray(S, np.float32)) * np.float32(1.0 / 25.0)
    out = np.empty((B, H, W, K, K), dtype=np.float32)
    for i in range(8):
        b, half = divmod(i, 2)
        hs = slice(half * HH, (half + 1) * HH)
        r = np.asarray(results[i]["R"]).astype(np.float32)   # [136, 128, 256]
        v = r - mu[b, iu, hs, :] * mu[b, il, hs, :]          # [136, 128, 256]
        v = np.moveaxis(v, 0, -1)                            # [128, 256, 136]
        flat = np.empty((HH, W, K * K), dtype=np.float32)
        flat[..., iu * K + il] = v
        flat[..., il * K + iu] = v
        out[b, hs] = flat.reshape(HH, W, K, K)
    return out


def kernel(S):
    """S: [4, 16, 256, 256] float32 -> R: [4, 256, 256, 16, 16] float32."""
    nc = _get_nc()
    in_maps = _prep_in_maps(S)
    res = bass_utils.run_bass_kernel_spmd(nc, in_maps, list(range(8)))
    return _assemble(res.results, S)


# revision 18
# speedup vs baseline: 1.0815x; 1.0087x over previous
"""Trainium2 Bass kernel: per-pixel 5x5-patch channel covariance.

R[b,h,w,k,l] = (1/N) sum_n (p_kn - mu_k)(p_ln - mu_l)   (N=25, reflect pad)

Identity:  R = box5x5(S_k * S_l)/25 - mu_k * mu_l,  mu = box5x5(S)/25.

Device computes ONLY the 136 upper-triangle pair channels box5x5(S_k*S_l)/25
(host pre-scales S by 1/5 so two weight-1 banded box passes give /25).
Host computes mu (cheap separable box in fp32), subtracts mu_k*mu_l, and
mirrors the symmetric lower triangle -- all trivially vectorized numpy.

Dataflow per core (shard = one batch x one H-half, 128 out rows + halo):
  products (DVE, 4x mode)
    -> stage-1 row-box banded matmuls (TensorE, out = [w_chunk, 64h] psum)
    -> psum->sbuf copies into i1[w, ch, h] (split Act/Pool)
    -> stage-2 col-box matmuls lhsT=i1[:,c,:], rhs=BW chunk (out = [128h, 256w])
    -> psum->sbuf copies (split DVE/Act/Pool)
    -> DMA out channel-major [136, 128, 256] (512B descriptors)

Sharding: 8 cores = 4 batches x 2 H-halves.  Fully data parallel.
"""
import sys

sys.path.insert(0, "/opt/trn_rl_repo")

from contextlib import ExitStack

import numpy as np

import concourse.bacc as bacc
import concourse.mybir as mybir
import concourse.tile as tile
from concourse import bass_utils

B, K, H, W = 4, 16, 256, 256
HH = 128           # output rows per core
SR = 132           # shard rows (128 + 2 halo each side, edge-clamped)
NPAIR = K * (K + 1) // 2   # 136 upper-triangle channels
NOCT = NPAIR // 8          # 17 channel octets
F32 = mybir.dt.float32
BF16 = mybir.dt.bfloat16

# Pool (GPSIMD) cannot read PSUM, so all psum->sbuf copies go Act/DVE
# (2:1 by rate); Pool instead takes the last product octets off DVE,
# emitted as a pre-pass so they are ready when the main loop reaches them.
# Ordered: emitted (and consumed) in this sequence; spread so one product
# octet per channel-group comes from Pool throughout the run.
POOL_OCTS = [(0, 1), (0, 3), (0, 5), (0, 7), (0, 9), (0, 11),
             (0, 13), (1, 13), (0, 15), (1, 15), (1, 16)]


def _reflect_idx(i, n):
    if i < 0:
        return -i
    if i >= n:
        return 2 * (n - 1) - i
    return i


def _build_bw():
    """[128, 512] col-box weights, reflect folded: [:, c*256:(c+1)*256] =
    M[c*128:(c+1)*128, :] where M[w_src, w_out] is the 256x256 band."""
    M = np.zeros((W, W), dtype=np.float32)
    for w in range(W):
        for j in range(5):
            M[_reflect_idx(w - 2 + j, W), w] += 1.0
    out = np.zeros((128, 512), dtype=np.float32)
    out[:, 0:256] = M[0:128, :]
    out[:, 256:512] = M[128:256, :]
    return out


def _build_br(half):
    """[68, 128] row-box weights: cols rt*64+hl; rows shard-local within rt."""
    hbase = half * HH
    M = np.zeros((68, 128), dtype=np.float32)
    for rt in range(2):
        for hl in range(64):
            hg = hbase + rt * 64 + hl
            for i in range(5):
                r = _reflect_idx(hg - 2 + i, H)
                j = r + 2 - hbase          # canonical shard row
                M[j - rt * 64, rt * 64 + hl] += 1.0
    return M


def _ksegs_in_octet(oct_idx):
    """Pair channels 0..135 in (k outer, l=k..15) order. For channel octet
    [oct*8, oct*8+8) return (j0, k, l0, nl): local offset, k, first l, count."""
    lo, hi = oct_idx * 8, oct_idx * 8 + 8
    segs = []
    p = 0
    for k in range(K):
        n = K - k
        s, e = p, p + n
        a, b = max(lo, s), min(hi, e)
        if a < b:
            segs.append((a - lo, k, k + (a - s), b - a))
        p += n
    return segs


def _build_kernel():
    nc = bacc.Bacc("TRN2", target_bir_lowering=False, debug=False)
    S_d = nc.dram_tensor("S", [SR, K, W], BF16, kind="ExternalInput").ap()
    BR_d = nc.dram_tensor("BR", [68, 128], BF16, kind="ExternalInput").ap()
    BW_d = nc.dram_tensor("BW", [128, 512], BF16, kind="ExternalInput").ap()
    R_d = nc.dram_tensor("R", [NPAIR, HH, W], BF16, kind="ExternalOutput").ap()

    cp_eng = None  # set inside

    with tile.TileContext(nc) as tc, ExitStack() as ctx:
        const_p = ctx.enter_context(tc.tile_pool(name="const", bufs=1))
        sp_p = ctx.enter_context(tc.tile_pool(name="sp", bufs=1))
        t_p = ctx.enter_context(tc.tile_pool(name="tprod", bufs=10))
        tp_p = ctx.enter_context(tc.tile_pool(name="tpool", bufs=1))
        i1_p = ctx.enter_context(tc.tile_pool(name="i1", bufs=1))
        r_p = ctx.enter_context(tc.tile_pool(name="rout", bufs=4))
        ps1_p = ctx.enter_context(tc.tile_pool(name="ps1", bufs=2, space="PSUM"))
        ps2_p = ctx.enter_context(tc.tile_pool(name="ps2", bufs=2, space="PSUM"))

        ncopy = 0

        def copy_psum(dst, src):
            nonlocal ncopy
            if ncopy % 3 == 2:
                nc.vector.tensor_copy(dst, src)
            else:
                nc.scalar.copy(dst, src)
            ncopy += 1

        br = const_p.tile([68, 128], BF16)
        bw = const_p.tile([128, 512], BF16)
        nc.sync.dma_start(br[:], BR_d)
        nc.sync.dma_start(bw[:], BW_d)

        sp0 = sp_p.tile([68, K, W], BF16)
        sp1 = sp_p.tile([68, K, W], BF16)
        # split loads so the first product octets can start sooner
        nc.sync.dma_start(sp0[:, 0:8, :], S_d[0:68, 0:8, :])
        nc.sync.dma_start(sp0[:, 8:16, :], S_d[0:68, 8:16, :])
        nc.sync.dma_start(sp1[:, 0:8, :], S_d[64:132, 0:8, :])
        nc.sync.dma_start(sp1[:, 8:16, :], S_d[64:132, 8:16, :])
        sps = [sp0, sp1]

        # i1[w_local, ch, chunk, h]: stage-1 output; chunk X covers w 128X..+128
        i1 = i1_p.tile([128, NPAIR, 2, HH], BF16, name="i1")

        def products(rt, oc, mul, pool):
            sp = sps[rt]
            T = pool.tile([68, 8, W], BF16, name=f"T{rt}_{oc}" if
                          pool is tp_p else "T")
            for (j0, k, l0, nl) in _ksegs_in_octet(oc):
                in0 = sp[:, k, :].unsqueeze(1).broadcast_to([68, nl, W])
                mul(T[:, j0:j0 + nl, :], in0, sp[:, l0:l0 + nl, :])
            return T

        # Pool pre-pass: spread product octets, in consumption order
        pool_T = {}
        for (rt, oc) in POOL_OCTS:
            pool_T[(rt, oc)] = products(rt, oc, nc.gpsimd.tensor_mul, tp_p)

        def prefetch_products(ocp):
            """Emit DVE products for group ocp ahead of older copies in the
            DVE queue, so products (critical path to PE) are not delayed."""
            if ocp > 8:
                return
            for oi in range(2 if ocp < 8 else 1):
                oc = ocp * 2 + oi
                for rt in range(2):
                    if (rt, oc) not in pool_T:
                        pool_T[(rt, oc)] = products(
                            rt, oc, nc.vector.tensor_mul, t_p)

        bw0 = bw[:, 0:256]
        bw1 = bw[:, 256:512]
        prefetch_products(0)
        for ocp in range(9):            # 16-channel groups (last is 8)
            nocts = 2 if ocp < 8 else 1
            prefetch_products(ocp + 1)
            # ---- stage 1: row box for this group's octets, both row-tiles
            for oi in range(nocts):
                oc = ocp * 2 + oi
                for rt in range(2):
                    brt = br[:, rt * 64:(rt + 1) * 64]
                    T = pool_T[(rt, oc)]
                    Tf = T[:].rearrange("p a b -> p (a b)")
                    ps1 = ps1_p.tile([128, 1024], F32, name="ps1")
                    for j in range(8):
                        nc.tensor.matmul(ps1[:, j * 128:j * 128 + 64],
                                         Tf[:, j * 256:j * 256 + 128], brt,
                                         start=True, stop=True)
                        nc.tensor.matmul(ps1[:, j * 128 + 64:j * 128 + 128],
                                         Tf[:, j * 256 + 128:j * 256 + 256],
                                         brt, start=True, stop=True)
                    copy_psum(
                        i1[:, oc * 8:(oc + 1) * 8, :, rt * 64:(rt + 1) * 64],
                        ps1[:].rearrange("p (c k h) -> p c k h", c=8, k=2))
            # ---- stage 2: col box -> [128 h, 256 w] per channel
            c0, nch = ocp * 16, 8 * nocts
            for cq in range(nch // 4):
                ps2 = ps2_p.tile([128, 1024], F32, name="ps2")
                for ci in range(4):
                    c = c0 + cq * 4 + ci
                    nc.tensor.matmul(ps2[:, ci * 256:(ci + 1) * 256],
                                     i1[:, c, 0, :], bw0,
                                     start=True, stop=False)
                    nc.tensor.matmul(ps2[:, ci * 256:(ci + 1) * 256],
                                     i1[:, c, 1, :], bw1,
                                     start=False, stop=True)
                rsb = r_p.tile([128, 4, W], BF16, name="rsb")
                copy_psum(rsb[:],
                          ps2[:].rearrange("p (c w) -> p c w", c=4))
                cb = c0 + cq * 4
                dview = R_d[cb:cb + 4, :, :].transpose([1, 0, 2])
                nc.sync.dma_start(dview, rsb[:])

    nc.compile()
    return nc


_NC_CACHE = {}


def _get_nc():
    if "nc" not in _NC_CACHE:
        _NC_CACHE["nc"] = _build_kernel()
    return _NC_CACHE["nc"]


def _prep_in_maps(S):
    S = np.asarray(S, dtype=np.float32)
    np_bf16 = mybir.dt.np(BF16)
    bw = _build_bw().astype(np_bf16)
    brs = [(_build_br(h)).astype(np_bf16) for h in range(2)]
    Ss = S * np.float32(0.2)
    in_maps = []
    for b in range(B):
        for half in range(2):
            hbase = half * HH
            rows = np.clip(np.arange(hbase - 2, hbase + 130), 0, H - 1)
            shard = Ss[b][:, rows, :].transpose(1, 0, 2)   # [132, K, 256]
            shard = np.ascontiguousarray(shard).astype(np_bf16)
            in_maps.append({"S": shard, "BR": brs[half], "BW": bw})
    return in_maps


def _box25(x):
    """Separable 5x5 box sum with reflect padding over last two axes."""
    xp = np.pad(x, ((0, 0), (0, 0), (2, 2), (2, 2)), mode="reflect")
    yh = xp[:, :, 0:H, :].copy()
    for i in range(1, 5):
        yh += xp[:, :, i:i + H, :]
    y = yh[:, :, :, 0:W].copy()
    for j in range(1, 5):
        y += yh[:, :, :, j:j + W]
    return y


def _assemble(results, S):
    iu, il = np.triu_indices(K)            # same order as device channels
    mu = _box25(np.asarray(S, np.float32)) * np.float32(1.0 / 25.0)
    out = np.empty((B, H, W, K, K), dtype=np.float32)
    for i in range(8):
        b, half = divmod(i, 2)
        hs = slice(half * HH, (half + 1) * HH)
        r = np.asarray(results[i]["R"]).astype(np.float32)   # [136, 128, 256]
        v = r - mu[b, iu, hs, :] * mu[b, il, hs, :]          # [136, 128, 256]
        v = np.moveaxis(v, 0, -1)                            # [128, 256, 136]
        flat = np.empty((HH, W, K * K), dtype=np.float32)
        flat[..., iu * K + il] = v
        flat[..., il * K + iu] = v
        out[b, hs] = flat.reshape(HH, W, K, K)
    return out


def kernel(S):
    """S: [4, 16, 256, 256] float32 -> R: [4, 256, 256, 16, 16] float32."""
    nc = _get_nc()
    in_maps = _prep_in_maps(S)
    res = bass_utils.run_bass_kernel_spmd(nc, in_maps, list(range(8)))
    return _assemble(res.results, S)
